# revision 5
# baseline (speedup 1.0000x reference)
"""Trainium2 Bass kernel v3 for nn_FEPSurrogateNetwork (GNN message passing).

Changes vs v1 baseline:
  * Gather of h[col] per 512-edge tile via ONE dma_gather(transpose=True)
    (SWDGE fixed cost paid once per tile, not 4x; output lands directly in
    [feature, edge] layout -> no PE transposes, no PSUM->SBUF copy).
    int16 gather indices can only address 32768 rows, so the node table is
    split in halves and edges are scheduled into col-half-homogeneous tiles
    (two interleaved row-window schedules, lo/hi).
  * dist term folded into the expand matmul (St gets a dist row, xw a wd row)
    -- removes a 512-row f32 matmul (~850ns PE) + a DMA per tile.
  * LayerNorm rstd batched per layer via one Rsqrt activation -- avoids 2
    ACT table reloads (2.6us) per 128-node block.
  * WIN=32 window schedule (G=20) shrinks St/S2 DMA bytes (fallback WIN=64).
  * v3: St+S2 merged into ONE DMA per tile ([128,640] image); xw windows
    computed on-PE from a zero-padded resident hT (kills phase A, the xloc
    DRAM roundtrip, and 2 of 3 per-tile DMAs -- the program was SP-sequencer
    and HWDGE bound at ~565-667ns per DMA instruction); gathers batched 2
    same-schedule tiles per dma_gather (1024 idxs).
"""

import os
from contextlib import ExitStack

import numpy as np
import ml_dtypes

import concourse.bacc as bacc
import concourse.bass as bass
import concourse.mybir as mybir
import concourse.tile as tile
from concourse.bass_utils import run_bass_kernel_spmd
from concourse.library_config import mlp as mlp_lib

BF16 = mybir.dt.bfloat16
F32 = mybir.dt.float32
I16 = mybir.dt.int16
AF = mybir.ActivationFunctionType
ALU = mybir.AluOpType
ts = bass.ts

P = 128


class Cfg:
    def __init__(self, N, E, HD, L, NC=8, G=20, CAP=128, WIN=32):
        # (G=20, WIN=32) primary; (G=12, WIN=56) fallback -- both SWIN=94
        assert HD == P
        self.N, self.E, self.HD, self.L, self.NC = N, E, HD, L, NC
        self.G, self.CAP, self.WIN = G, CAP, WIN
        self.SWIN = 3 * G + WIN + 2
        self.SWPAD = (self.SWIN + 31) // 32 * 32  # DVE partition alignment
        self.SROWS = self.SWPAD + 1  # + wd/dist row at aligned partition
        assert self.SROWS <= P
        self.NLOC = N // NC
        assert self.NLOC * NC == N
        self.NB = (self.NLOC + P - 1) // P
        self.NLOCP = self.NB * P
        self.HALF = (NC // 2) * self.NLOCP  # gid half boundary
        assert self.HALF < 32768
        ch = (self.NLOC + G - 1) // G + 3
        self.CHUNKS = (ch + 3) // 4 * 4  # per schedule
        self.TPS = self.CHUNKS // 4  # tiles per schedule
        self.TILES = 2 * self.TPS  # merged (lo/hi interleaved)
        self.o = lambda c: (c + 1) * G - WIN
        self.sbase = lambda t: 4 * G * t + G - WIN - 2
        self.NSW = (self.o(self.CHUNKS - 1) + WIN + 511) // 512
        self.XPAD = WIN + 2
        self.XROWS = self.XPAD + self.NLOCP + self.SWPAD + 66
        self.NSTRIP = (self.NB + 3) // 4  # 512-wide node strips


def g_cfg(variant=0):
    g, win = [(20, 32), (12, 56)][variant]
    return Cfg(N=50000, E=600000, HD=128, L=4, G=g, WIN=win)


# ---------------------------------------------------------------- host prep


def _greedy_chunks(cfg, r, cl, dd):
    """Schedule one half's edges (sorted by local row r) into CHUNKS chunks.
    Returns (rows, cols, dist, valid) arrays [CHUNKS, CAP]. Raises
    AssertionError if WIN is infeasible."""
    G, CAP, WIN, CHUNKS, NLOC = cfg.G, cfg.CAP, cfg.WIN, cfg.CHUNKS, cfg.NLOC
    ch_rows = np.zeros((CHUNKS, CAP), np.int64)
    ch_cols = np.zeros((CHUNKS, CAP), np.int64)
    ch_dist = np.zeros((CHUNKS, CAP), np.float32)
    ch_valid = np.zeros((CHUNKS, CAP), bool)
    cur = 0
    for k in range(CHUNKS):
        node_end = min((k + 1) * G, NLOC)
        n_avail = int(np.searchsorted(r, node_end, side="left")) - cur
        take = min(CAP, n_avail)
        if take > 0:
            sl = slice(cur, cur + take)
            assert r[sl].min() >= max(0, cfg.o(k)), "schedule infeasible"
            ch_rows[k, :take] = r[sl]
            ch_cols[k, :take] = cl[sl]
            ch_dist[k, :take] = dd[sl]
            ch_valid[k, :take] = True
            cur += take
    assert cur == len(r), "edges unassigned; schedule infeasible"
    return ch_rows, ch_cols, ch_dist, ch_valid


def host_prep(cfg, z, pos, edge_index, lam, atom_embed, lam_w, lam_b):
    N, NC, G, CAP, WIN, SWIN = cfg.N, cfg.NC, cfg.G, cfg.CAP, cfg.WIN, cfg.SWIN
    NLOC, NLOCP, CHUNKS, TPS = cfg.NLOC, cfg.NLOCP, cfg.CHUNKS, cfg.TPS
    bf16 = ml_dtypes.bfloat16

    z = np.asarray(z)
    pos = np.asarray(pos, np.float32)
    ei = np.asarray(edge_index)
    lam = np.float32(np.asarray(lam))
    atom_embed = np.asarray(atom_embed, np.float32)
    lam_vec = lam * np.asarray(lam_w, np.float32)[0] + np.asarray(lam_b, np.float32)
    h0 = atom_embed[z] + lam_vec[None, :]

    row, col = ei[0].astype(np.int64), ei[1].astype(np.int64)
    diff = pos[row] - pos[col]
    dist = np.sqrt((diff * diff).sum(-1) + 1e-8).astype(np.float32)

    gid = (col // NLOC) * NLOCP + (col % NLOC)  # padded-global col id

    order = np.argsort(row, kind="stable")
    row_s, gid_s, dist_s = row[order], gid[order], dist[order]
    core_of = row_s // NLOC

    out = dict(idx=[], SS2=[], h0slice=[], h0sliceT=[])
    for c in range(NC):
        m = core_of == c
        r_all = row_s[m] - c * NLOC
        g_all = gid_s[m]
        d_all = dist_s[m]
        half_of = (g_all >= cfg.HALF).astype(np.int64)

        sched = []
        for s in range(2):
            hm = half_of == s
            sched.append(
                _greedy_chunks(cfg, r_all[hm], g_all[hm] - s * cfg.HALF, d_all[hm])
            )

        # per merged tile tt = 2*t + s; SS2 = [St | S2] packed [128, 640]
        idx_all = np.zeros((P, cfg.TILES * (4 * CAP // 16)), np.int16)
        SS2_all = np.zeros((cfg.TILES, P, 4 * CAP + 4 * WIN), bf16)
        for t in range(TPS):
            for s in range(2):
                tt = 2 * t + s
                ch_rows, ch_cols, ch_dist, ch_valid = sched[s]
                base = max(0, cfg.sbase(t))
                idxs = np.zeros((4 * CAP,), np.int16)
                for j in range(4):
                    k = 4 * t + j
                    v = ch_valid[k]
                    e_off = j * CAP + np.arange(CAP)
                    # gather indices (pad -> 0)
                    idxs[e_off] = np.where(v, ch_cols[k], 0).astype(np.int16)
                    # expand one-hot (+ dist row SWPAD)
                    rel = ch_rows[k] - base
                    assert np.all((rel[v] >= 0) & (rel[v] < cfg.SWPAD))
                    SS2_all[tt, rel[v], e_off[v]] = 1.0
                    SS2_all[tt, cfg.SWPAD, e_off] = np.where(v, ch_dist[k], 0.0).astype(bf16)
                    # scatter one-hot (packed after the St block)
                    rel2 = ch_rows[k] - cfg.o(k)
                    assert np.all((rel2[v] >= 0) & (rel2[v] < WIN))
                    SS2_all[tt, np.arange(CAP)[v], 4 * CAP + j * WIN + rel2[v]] = 1.0
                # wrap int16 idxs: position i -> [i % 16, i // 16], replicate
                # x8; paired-gather layout: tile t of schedule s at s*TPS + t
                w = idxs.reshape(4 * CAP // 16, 16).T  # [16, 32]
                slot = s * TPS + t
                idx_all[:, slot * (4 * CAP // 16) : (slot + 1) * (4 * CAP // 16)] = (
                    np.tile(w, (8, 1))
                )
        out["idx"].append(idx_all)
        out["SS2"].append(np.ascontiguousarray(SS2_all))

        hs = np.zeros((NLOCP, cfg.HD), np.float32)
        hs[:NLOC] = h0[c * NLOC : (c + 1) * NLOC]
        out["h0slice"].append(hs.astype(bf16))
        out["h0sliceT"].append(np.ascontiguousarray(hs.T).astype(bf16))

    # full padded-global h0 table (replicated input; skips the initial AllGather)
    hf = np.zeros((NC * NLOCP, cfg.HD), np.float32)
    for c in range(NC):
        hf[c * NLOCP : c * NLOCP + NLOC] = h0[c * NLOC : (c + 1) * NLOC]
    hfull0 = hf.astype(bf16)
    for c in range(NC):
        out.setdefault("h0full", []).append(hfull0)

    return out, h0


def host_weights(cfg, lam, lam_w, lam_b, edge_w1, edge_b1, edge_w2, edge_b2,
                 node_w1, node_b1, node_w2, node_b2, ln_g, ln_b,
                 head_w1, head_b1, head_w2, head_b2):
    f = lambda x: np.asarray(x, np.float32)
    bf16 = ml_dtypes.bfloat16
    lam = np.float32(np.asarray(lam))
    lam_vec = lam * f(lam_w)[0] + f(lam_b)
    W = {}
    HD = cfg.HD
    for i in range(cfg.L):
        w1 = f(edge_w1[i])  # [3*HD+1, HD]
        W[f"W1a_{i}"] = w1[:HD].astype(bf16)
        W[f"W1b_{i}"] = w1[HD : 2 * HD].astype(bf16)
        W[f"wd_{i}"] = np.ascontiguousarray(w1[2 * HD : 2 * HD + 1]).astype(bf16)  # [1,HD]
        b1e = f(edge_b1[i]) + lam_vec @ w1[2 * HD + 1 :]
        W[f"b1_{i}"] = b1e[:, None].astype(np.float32)  # [HD,1]
        W[f"W2_{i}"] = f(edge_w2[i]).astype(bf16)
        W[f"b2r_{i}"] = f(edge_b2[i])[None, :].astype(np.float32)
        nw1 = f(node_w1[i])
        W[f"nw1a_{i}"] = nw1[:HD].astype(bf16)
        W[f"nw1b_{i}"] = nw1[HD:].astype(bf16)
        W[f"nb1_{i}"] = f(node_b1[i])[:, None].astype(np.float32)
        W[f"nw2_{i}"] = f(node_w2[i]).astype(bf16)
        W[f"nb2r_{i}"] = f(node_b2[i])[None, :].astype(np.float32)
        W[f"g_{i}"] = np.broadcast_to(f(ln_g[i])[None, :], (P, HD)).astype(bf16).copy()
        W[f"b_{i}"] = np.broadcast_to(f(ln_b[i])[None, :], (P, HD)).astype(bf16).copy()
    W["hw1"] = f(head_w1)
    W["hb1r"] = f(head_b1)[None, :]
    W["hw2"] = f(head_w2)
    W["hb2"] = f(head_b2)[None, :]
    mask = np.zeros((cfg.NB * P,), np.float32)
    mask[: cfg.NLOC] = 1.0
    W["pmask"] = np.ascontiguousarray(mask.reshape(cfg.NB, P).T).astype(bf16)
    triv = {
        "b2": all(not np.any(f(edge_b2[i])) for i in range(cfg.L)),
        "nb2": all(not np.any(f(node_b2[i])) for i in range(cfg.L)),
        "gb": all(
            np.all(f(ln_g[i]) == 1.0) and not np.any(f(ln_b[i])) for i in range(cfg.L)
        ),
        "hb1": not np.any(f(head_b1)),
        "hb2": not np.any(f(head_b2)),
    }
    return W, triv


# ------------------------------------------------------------- device program


def scatter_writes(cfg):
    """Program-order list of scatter matmul writes [(sw, tt, j, lo, hi)] and
    per-superwindow (first_tt, last_index)."""
    writes = []
    for tt in range(cfg.TILES):
        t = tt // 2
        for j in range(4):
            c = 4 * t + j
            a0, b0 = max(0, cfg.o(c)), cfg.o(c) + cfg.WIN
            for s in range(cfg.NSW):
                lo, hi = max(a0, 512 * s), min(b0, 512 * (s + 1))
                if lo < hi:
                    writes.append((s, tt, j, lo, hi))
    first_tt, last_idx = {}, {}
    for i, (s, tt, j, lo, hi) in enumerate(writes):
        if s not in first_tt:
            first_tt[s] = tt
        last_idx[s] = i
    return writes, first_tt, last_idx


def build_program(cfg, triv, skip=()):
    skip = set(skip)
    NB, TILES, WIN, SWIN = cfg.NB, cfg.TILES, cfg.WIN, cfg.SWIN
    L, NLOCP, NC, NSTRIP = cfg.L, cfg.NLOCP, cfg.NC, cfg.NSTRIP
    CAP = cfg.CAP
    IDXW = 4 * CAP // 16  # idx cols per tile
    nc = bacc.Bacc("TRN2", debug=False, num_devices=NC, target_bir_lowering=False,
               num_swdge_queues=4)

    d_idx = nc.dram_tensor("idx", [P, TILES * IDXW], I16, kind="ExternalInput")
    d_SS2 = nc.dram_tensor(
        "SS2", [TILES, P, 4 * CAP + 4 * WIN], BF16, kind="ExternalInput"
    )
    d_h0 = nc.dram_tensor("h0slice", [NLOCP, P], BF16, kind="ExternalInput")
    d_h0full = nc.dram_tensor("h0full", [NC * NLOCP, P], BF16, kind="ExternalInput")
    d_h0T = nc.dram_tensor("h0sliceT", [P, NLOCP], BF16, kind="ExternalInput")
    wnames = (
        [f"{n}_{i}" for i in range(L) for n in
         ("W1a", "W1b", "W2", "nw1a", "nw1b", "nw2", "g", "b")]
    )
    d_w = {n: nc.dram_tensor(n, [P, P], BF16, kind="ExternalInput") for n in wnames}
    for i in range(L):
        d_w[f"wd_{i}"] = nc.dram_tensor(f"wd_{i}", [1, P], BF16, kind="ExternalInput")
        for n, shp in ((f"b1_{i}", [P, 1]), (f"nb1_{i}", [P, 1]),
                       (f"b2r_{i}", [1, P]), (f"nb2r_{i}", [1, P])):
            d_w[n] = nc.dram_tensor(n, shp, F32, kind="ExternalInput")
    d_w["hw1"] = nc.dram_tensor("hw1", [P, P], F32, kind="ExternalInput")
    d_w["hb1r"] = nc.dram_tensor("hb1r", [1, P], F32, kind="ExternalInput")
    d_w["hw2"] = nc.dram_tensor("hw2", [P, 1], F32, kind="ExternalInput")
    d_w["hb2"] = nc.dram_tensor("hb2", [1, 1], F32, kind="ExternalInput")
    d_w["pmask"] = nc.dram_tensor("pmask", [P, NB], BF16, kind="ExternalInput")
    d_out = nc.dram_tensor("out_y", [1, 1], F32, kind="ExternalOutput")

    d_hslice = nc.dram_tensor("hslice", [NLOCP, P], BF16)
    d_hfull = nc.dram_tensor("hfull", [NC * NLOCP, P], BF16, addr_space="Shared")
    d_pool = nc.dram_tensor("poolpart", [1, P], F32)
    d_poolr = nc.dram_tensor("poolred", [1, P], F32, addr_space="Shared")

    groups = [list(range(NC))]
    writes, sw_first_tt, sw_last_idx = scatter_writes(cfg)
    # index writes by tile for emission
    writes_by_tt = {}
    for i, wr in enumerate(writes):
        writes_by_tt.setdefault(wr[1], []).append((i, wr))

    with ExitStack() as ctx:
        tc = ctx.enter_context(tile.TileContext(nc))
        cst = ctx.enter_context(tc.tile_pool(name="cst", bufs=1))
        res = ctx.enter_context(tc.tile_pool(name="res", bufs=1))
        xs = ctx.enter_context(tc.tile_pool(name="xs", bufs=1))
        sbe = ctx.enter_context(tc.tile_pool(name="sbe", bufs=6))
        sbn = ctx.enter_context(tc.tile_pool(name="sbn", bufs=4))
        ps = ctx.enter_context(tc.tile_pool(name="ps", bufs=2, space="PSUM"))
        pagg = ctx.enter_context(tc.tile_pool(name="pagg", bufs=2, space="PSUM"))

        nc.gpsimd.load_library(mlp_lib)
        nidx_reg = nc.gpsimd.to_reg(8 * CAP)
        nidx1_reg = nc.gpsimd.to_reg(4 * CAP)

        # ---- resident constants
        idxall = cst.tile([P, TILES * IDXW], I16)
        nc.sync.dma_start(idxall[:], d_idx[:])
        onesf = cst.tile([1, 1], F32)
        nc.vector.memset(onesf[:], 1.0)
        onesf_col = cst.tile([1, P], F32)
        nc.vector.memset(onesf_col[:], 1.0)
        zrow = cst.tile([1, 4 * P], BF16)
        nc.vector.memset(zrow[:], 0.0)
        zrow128 = cst.tile([P, P], BF16)
        nc.vector.memset(zrow128[:], 0.0)
        pmask = cst.tile([P, NB], BF16)
        nc.sync.dma_start(pmask[:], d_w["pmask"][:])
        eps_t = cst.tile([P, 1], F32)
        nc.vector.memset(eps_t[:], 1e-5)
        ident = cst.tile([P, P], BF16)
        from concourse.masks import make_identity
        make_identity(nc, ident[:])

        wt = {}
        for n in wnames:
            wt[n] = cst.tile([P, P], BF16, tag=f"w_{n}", name=f"w_{n}")
            nc.sync.dma_start(wt[n][:], d_w[n][:])
        for i in range(L):
            for n, shp, dt in (
                (f"wd_{i}", [1, P], BF16),
                (f"b1_{i}", [P, 1], F32),
                (f"nb1_{i}", [P, 1], F32),
                (f"b2r_{i}", [1, P], F32),
                (f"nb2r_{i}", [1, P], F32),
            ):
                wt[n] = cst.tile(shp, dt, tag=f"w_{n}", name=f"w_{n}")
                nc.sync.dma_start(wt[n][:], d_w[n][:])
        for n, shp in (("hw1", [P, P]), ("hb1r", [1, P]), ("hw2", [P, 1]), ("hb2", [1, 1])):
            wt[n] = cst.tile(shp, F32, tag=f"w_{n}", name=f"w_{n}")
            nc.sync.dma_start(wt[n][:], d_w[n][:])

        # resident state (hT padded with SWPAD zero cols for window reads)
        hTx = res.tile([P, NLOCP + cfg.SWPAD], BF16)
        nc.vector.memset(hTx[:], 0.0)
        hT = hTx[:, :NLOCP]
        hnat = res.tile([P, NB * P], BF16)
        aggT = res.tile([P, NLOCP], BF16)
        nc.vector.memset(aggT[:], 0.0)
        nc.sync.dma_start(hT, d_h0T[:])
        for b in range(NB):
            nc.sync.dma_start(hnat[:, ts(b, P)], d_h0[b * P : (b + 1) * P, :])
        # LN stats tiles
        negmu_all = res.tile([P, NB], F32)
        ssq_all = res.tile([P, NB], F32)
        rstd_all = res.tile([P, NB], F32)


        def strip_cols(si):
            w = min(4 * P, NB * P - si * 4 * P)
            return si * 4 * P, w

        for li in range(L):
            W1a, W1b, W2 = wt[f"W1a_{li}"], wt[f"W1b_{li}"], wt[f"W2_{li}"]
            wd, b1 = wt[f"wd_{li}"], wt[f"b1_{li}"]

            nw1a, nw1b, nb1, nw2 = (
                wt[f"nw1a_{li}"], wt[f"nw1b_{li}"], wt[f"nb1_{li}"], wt[f"nw2_{li}"]
            )
            last = li == L - 1
            x_strips = {}
            pass2_done = set()

            def phaseC_pass2(si):
                c0, w = strip_cols(si)
                nbk = w // P
                x = x_strips[si]
                hTps = ps.tile([P, 4 * P], BF16, tag="scr")
                for j in range(nbk):
                    b = si * 4 + j
                    ydst = hnat[:, c0 + j * P : c0 + (j + 1) * P]
                    if triv["gb"]:
                        nc.vector.tensor_scalar(
                            ydst, x[:, ts(j, P)],
                            scalar1=negmu_all[:, b : b + 1],
                            scalar2=rstd_all[:, b : b + 1],
                            op0=ALU.add, op1=ALU.mult,
                        )
                    else:
                        y = sbn.tile([P, P], BF16, tag="y")
                        nc.vector.tensor_scalar(
                            y[:], x[:, ts(j, P)],
                            scalar1=negmu_all[:, b : b + 1],
                            scalar2=rstd_all[:, b : b + 1],
                            op0=ALU.add, op1=ALU.mult,
                        )
                        yg = sbn.tile([P, P], BF16, tag="yg")
                        nc.vector.tensor_tensor(
                            out=yg[:], in0=y[:], in1=wt[f"g_{li}"][:], op=ALU.mult
                        )
                        nc.vector.tensor_tensor(
                            out=ydst, in0=yg[:], in1=wt[f"b_{li}"][:], op=ALU.add
                        )
                    nc.tensor.transpose(hTps[:, ts(j, P)], ydst, ident[:])
                    if last:
                        nc.tensor.matmul(
                            agg_pool[:], pmask[:, b : b + 1],
                            hnat[:, c0 + j * P : c0 + (j + 1) * P],
                            start=(b == 0), stop=(b == NB - 1),
                        )
                nc.vector.tensor_copy(hTx[:, c0 : c0 + w], hTps[:, :w])
                if not last and "cc" not in skip:
                    nc.sync.dma_start(
                        d_hslice[c0 : c0 + w, :].rearrange("(j p) f -> p j f", p=P),
                        hnat[:, c0 : c0 + w].rearrange("p (j f) -> p j f", f=P),
                    )

            def phaseC_pass1(si):
                c0, w = strip_cols(si)
                nbk = w // P
                u1 = ps.tile([P, 4 * P], F32, tag="m1")
                for j in range(nbk):
                    cj = slice(c0 + j * P, c0 + (j + 1) * P)
                    nc.tensor.matmul(u1[:, ts(j, P)], nw1a[:], hTx[:, cj],
                                     start=True, stop=False)
                    nc.tensor.matmul(u1[:, ts(j, P)], nw1b[:], aggT[:, cj],
                                     start=False, stop=True)
                u1sb = sbn.tile([P, 4 * P], BF16, tag="u1sb")
                nc.scalar.activation(u1sb[:, :w], u1[:, :w], AF.Silu, bias=nb1[:])
                u2 = ps.tile([P, 4 * P], F32, tag="m2")
                for j in range(nbk):
                    nc.tensor.matmul(
                        u2[:, ts(j, P)], u1sb[:, ts(j, P)], nw2[:],
                        start=True, stop=triv["nb2"],
                    )
                    if not triv["nb2"]:
                        nc.tensor.matmul(
                            u2[:, ts(j, P)], onesf_col[:1, :], wt[f"nb2r_{li}"][:1, :],
                            start=False, stop=True,
                        )
                x = xs.tile([P, 4 * P], F32, tag=f"x_{si}")
                x_strips[si] = x
                nc.vector.tensor_tensor(
                    out=x[:, :w], in0=hnat[:, c0 : c0 + w], in1=u2[:, :w], op=ALU.add
                )
                for j in range(nbk):
                    b = si * 4 + j
                    red = sbn.tile([P, 1], F32, tag="red")
                    nc.vector.tensor_reduce(
                        red[:], x[:, ts(j, P)], axis=mybir.AxisListType.X, op=ALU.add
                    )
                    nc.vector.tensor_scalar_mul(
                        negmu_all[:, b : b + 1], red[:], -1.0 / P
                    )
                    xsq = ps.tile([P, P], F32, tag="scr")
                    nc.scalar.activation(
                        xsq[:], x[:, ts(j, P)], AF.Square,
                        bias=negmu_all[:, b : b + 1],
                        accum_out=ssq_all[:, b : b + 1],
                    )

            # ---- phase B: edge tiles (merged lo/hi schedules)
            agg_tiles = {}
            xw_cur = None
            gt_cur = [None, None]
            for tt in range(TILES if "edge" not in skip else 0):
                s_half = tt % 2
                t = tt // 2
                # arm superwindows
                for s in range(cfg.NSW):
                    if sw_first_tt.get(s) == tt:
                        ag = pagg.tile([P, 512], F32, tag="agg")
                        agg_tiles[s] = ag
                        nc.tensor.matmul(
                            ag[:], zrow[:1, :P], zrow[:1, :], start=True, stop=False
                        )

                if s_half == 0:
                    # window xw = (h @ W1a)[XB:XB+SWPAD] computed on PE,
                    # shared by this lo tile and the next hi tile
                    XB = max(0, cfg.sbase(t))
                    xwp = ps.tile([P, P], F32, tag="scr")
                    nc.tensor.matmul(
                        xwp[: cfg.SWPAD, :], hTx[:, XB : XB + cfg.SWPAD],
                        W1a[:], start=True, stop=True,
                    )
                    xw_cur = sbe.tile([cfg.SROWS, P], BF16, tag="xw")
                    nc.vector.tensor_copy(xw_cur[: cfg.SWPAD, :], xwp[: cfg.SWPAD, :])
                    nc.vector.tensor_copy(xw_cur[cfg.SWPAD : cfg.SROWS, :], wd[:])
                xw = xw_cur

                ss2 = sbe.tile([P, 4 * CAP + 4 * WIN], BF16, tag="ss2")
                nc.sync.dma_start(ss2[:], d_SS2[tt, :, :])

                if "gather" not in skip:
                    PAIR = os.environ.get("K_PAIR", "0") == "1"
                    if not PAIR or t % 2 == 0:
                        # one dma_gather covers schedule tiles t, t+1
                        npair = (2 if t + 1 < cfg.TPS else 1) if PAIR else 1
                        gt2 = sbe.tile([P, npair * 4 * P], BF16, tag=f"gt{s_half}")
                        hsrc = d_h0full if li == 0 else d_hfull
                        hview = (
                            hsrc[: cfg.HALF, :]
                            if s_half == 0
                            else hsrc[cfg.HALF : 2 * cfg.HALF, :]
                        )
                        slot = s_half * cfg.TPS + t
                        nc.gpsimd.dma_gather(
                            out_ap=gt2[:].rearrange("p (o i) -> p o i", o=1),
                            in_ap=hview,
                            idxs_ap=idxall[:, slot * IDXW : (slot + npair) * IDXW],
                            num_idxs=npair * 4 * CAP,
                            num_idxs_reg=nidx_reg if npair == 2 else nidx1_reg,
                            elem_size=P,
                            transpose=True,
                            queue_num=tt % 4,
                        )
                        gt_cur[s_half] = gt2
                    toff = (t % 2) * 4 * P if PAIR else 0
                    gt = gt_cur[s_half][:, toff : toff + 4 * P]
                else:
                    gtz = sbe.tile([P, 4 * P], BF16, tag=f"gt{s_half}")
                    nc.vector.memset(gtz[:], 0.0)
                    gt = gtz[:]

                m1 = ps.tile([P, 4 * P], F32, tag="m1")
                nc.tensor.matmul(
                    m1[:], xw[:], ss2[: cfg.SROWS, : 4 * CAP], start=True, stop=False
                )
                nc.tensor.matmul(m1[:], W1b[:], gt, start=False, stop=True)
                m1sb = sbe.tile([P, 4 * P], BF16, tag="m1sb")
                nc.scalar.activation(m1sb[:], m1[:], AF.Silu, bias=b1[:])

                m2 = ps.tile([P, 4 * P], F32, tag="m2")
                for j in range(4):
                    nc.tensor.matmul(
                        m2[:, ts(j, P)], m1sb[:, ts(j, P)], W2[:],
                        start=True, stop=triv["b2"],
                    )
                    if not triv["b2"]:
                        nc.tensor.matmul(
                            m2[:, ts(j, P)], onesf_col[:1, :], wt[f"b2r_{li}"][:1, :],
                            start=False, stop=True,
                        )
                msb = sbe.tile([P, 4 * P], BF16, tag="msb")
                nc.scalar.activation(msb[:], m2[:], AF.Silu)

                if "scatter" not in skip:
                    s2t = ss2[:, 4 * CAP :]
                    for i, (s, tt_, j, lo, hi) in writes_by_tt.get(tt, []):
                        c = 4 * t + j
                        nc.tensor.matmul(
                            agg_tiles[s][:, lo - 512 * s : hi - 512 * s],
                            msb[:, ts(j, P)],
                            s2t[:, j * WIN + lo - cfg.o(c) : j * WIN + hi - cfg.o(c)],
                            start=False,
                            stop=(i == sw_last_idx[s]),
                        )
                    # flush superwindows ending at this tile; immediately
                    # emit the node-MLP pass-1 for the freed 512-node strip so
                    # it overlaps the remaining (gather-bound) edge tiles
                    for s in range(cfg.NSW):
                        if s in agg_tiles and writes[sw_last_idx[s]][1] == tt:
                            wdt = min(512, NLOCP - 512 * s)
                            if wdt > 0:
                                nc.vector.tensor_copy(
                                    aggT[:, 512 * s : 512 * s + wdt],
                                    agg_tiles[s][:, :wdt],
                                )
                            del agg_tiles[s]
                            if s < NSTRIP and "node" not in skip:
                                phaseC_pass1(s)
                                # early normalize+export for strips 0..7 once
                                # their stats exist (remaining tiles read hTx
                                # cols >= 4626 only -- no overlap with writes
                                # to cols < 4096)
                                if s in (8, 11) and not last:
                                    blo, bhi = (0, 32) if s == 8 else (32, 48)
                                    sd_e = sbn.tile([P, NB], F32, tag="sd")
                                    nc.scalar.activation(
                                        sd_e[:, blo:bhi], ssq_all[:, blo:bhi],
                                        AF.Sqrt, scale=1.0 / P, bias=eps_t[:],
                                    )
                                    nc.vector.reciprocal(
                                        rstd_all[:, blo:bhi], sd_e[:, blo:bhi]
                                    )
                                    for si_e in range(blo // 4, bhi // 4):
                                        phaseC_pass2(si_e)
                                        pass2_done.add(si_e)

            # ---- phase C pass-2 (pass-1 strips were emitted at flush time)
            if "node" not in skip:
                for si in range(NSTRIP):
                    if si not in x_strips:  # e.g. under skip=edge
                        phaseC_pass1(si)
                if last:
                    agg_pool = pagg.tile([1, P], F32, tag="agg")
                tail = [si for si in range(NSTRIP) if si not in pass2_done]
                b0 = tail[0] * 4
                nbl = NB - b0
                sd_all = sbn.tile([P, NB], F32, tag="sd")
                nc.scalar.activation(
                    sd_all[:, b0 : b0 + nbl], ssq_all[:, b0 : b0 + nbl],
                    AF.Sqrt, scale=1.0 / P, bias=eps_t[:],
                )
                nc.vector.reciprocal(
                    rstd_all[:, b0 : b0 + nbl], sd_all[:, b0 : b0 + nbl]
                )
                for si in tail:
                    phaseC_pass2(si)

            if not last and "cc" not in skip:
                nc.gpsimd.collective_compute(
                    "AllGather", ALU.bypass, replica_groups=groups,
                    ins=[d_hslice[:]], outs=[d_hfull[:]],
                )

        # ---- head
        pool_sb = sbn.tile([1, P], F32, tag="pool_sb")
        nc.vector.tensor_scalar_mul(pool_sb[:], agg_pool[:], 1.0 / cfg.N)
        nc.sync.dma_start(d_pool[:], pool_sb[:])
        nc.gpsimd.collective_compute(
            "AllReduce", ALU.add, replica_groups=groups,
            ins=[d_pool[:]], outs=[d_poolr[:]],
        )
        pT = sbn.tile([P, 1], F32, tag="pT")
        nc.sync.dma_start(pT[:], d_poolr.rearrange("o d -> d o"))
        p1 = ps.tile([1, P], F32, tag="m1")
        nc.tensor.matmul(p1[:], pT[:], wt["hw1"][:], start=True, stop=triv["hb1"])
        if not triv["hb1"]:
            nc.tensor.matmul(p1[:], onesf[:], wt["hb1r"][:], start=False, stop=True)
        p1sb = sbn.tile([1, P], F32, tag="p1sb")
        nc.scalar.activation(p1sb[:], p1[:], AF.Silu)
        p1T = ps.tile([P, 1], F32, tag="m2")
        nc.tensor.matmul(p1T[:], p1sb[:], onesf[:], start=True, stop=True)
        p1T_sb = sbn.tile([P, 1], F32, tag="p1T_sb")
        nc.vector.tensor_copy(p1T_sb[:], p1T[:])
        yps = ps.tile([1, 1], F32, tag="scr")
        nc.tensor.matmul(yps[:], p1T_sb[:], wt["hw2"][:], start=True, stop=triv["hb2"])
        if not triv["hb2"]:
            nc.tensor.matmul(yps[:], onesf[:], wt["hb2"][:], start=False, stop=True)
        ysb = sbn.tile([1, 1], F32, tag="ysb")
        nc.vector.tensor_copy(ysb[:], yps[:])
        nc.sync.dma_start(d_out[:], ysb[:])

    nc.compile()
    return nc


# ------------------------------------------------------------------ kernel()

_cache = {}


def _get_program(cfg, triv_key, triv):
    skip = tuple(x for x in os.environ.get("K_SKIP", "").split(",") if x)
    key = (cfg.N, cfg.E, cfg.L, cfg.WIN, cfg.G, triv_key, skip, os.environ.get("K_PAIR", "0"))
    if key not in _cache:
        _cache[key] = build_program(cfg, triv, skip=skip)
    return _cache[key]


def prepare(inputs):
    last_err = None
    for variant in (0, 1):
        cfg = g_cfg(variant)
        try:
            data, _ = host_prep(
                cfg,
                inputs["z"], inputs["pos"], inputs["edge_index"], inputs["lam"],
                inputs["atom_embed"], inputs["lam_w"], inputs["lam_b"],
            )
            break
        except AssertionError as e:
            last_err = e
    else:
        raise last_err
    W, triv = host_weights(
        cfg, inputs["lam"], inputs["lam_w"], inputs["lam_b"],
        inputs["edge_w1"], inputs["edge_b1"], inputs["edge_w2"], inputs["edge_b2"],
        inputs["node_w1"], inputs["node_b1"], inputs["node_w2"], inputs["node_b2"],
        inputs["ln_g"], inputs["ln_b"],
        inputs["head_w1"], inputs["head_b1"], inputs["head_w2"], inputs["head_b2"],
    )
    triv_key = tuple(sorted(triv.items()))
    nc = _get_program(cfg, triv_key, triv)

    in_maps = []
    for c in range(cfg.NC):
        m = {
            "idx": data["idx"][c],
            "SS2": data["SS2"][c],
            "h0slice": data["h0slice"][c],
            "h0sliceT": data["h0sliceT"][c],
            "h0full": data["h0full"][c],
        }
        for k, v in W.items():
            m[k] = v
        in_maps.append(m)
    return nc, in_maps, cfg


def kernel(**inputs) -> np.ndarray:
    nc, in_maps, cfg = prepare(inputs)
    res = run_bass_kernel_spmd(nc, in_maps, core_ids=list(range(cfg.NC)))
    return res.results[0]["out_y"].reshape(1, 1).astype(np.float32)


if __name__ == "__main__":
    import sys
    sys.path.insert(0, "/root/problem")
    d = np.load("/root/problem/_inputs_cache.npz")
    inp = {k: d[k] for k in d.files if k != "_expected"}
    exp = d["_expected"].ravel()[0]
    got = kernel(**inp)
    rel = abs(got.ravel()[0] - exp) / (abs(exp) + 1e-12)
    print(f"expected {exp:.6e}  got {got.ravel()[0]:.6e}  rel {rel:.3e}")
